# revision 1
# baseline (speedup 1.0000x reference)
"""Trainium2 Bass kernel for 4-layer bidirectional GRU (H=128, T=200) + MLP head.

Strategy: data-parallel over the 400 flattened sequences -> 50 per core on 8
cores. On each core, all gate/state tiles use layout (128 partitions = hidden
unit, free dim = batch slots [fwd 50 | bwd 50]).

Per layer:
  - "precompute": input projections gi = Wih @ x (+bias) for chunks of CT
    timesteps as large matmuls (K=2x128 fp16), evicted PSUM->SBUF via the
    scalar engine with the per-partition bias folded into the activation's
    bias operand.
  - "scan": 200 sequential steps; recurrent matmuls (fp16 weights, FWL) with
    the precomputed gi pre-loaded into PSUM via an identity matmul so gate
    pre-activations come out of PSUM ready for the sigmoid/tanh activations.

Inter-layer activations are stored fp16 in a (128, T*100) SBUF buffer whose
block s holds [fwd output at time s | bwd output at scan step s] so the scan
reads/writes are contiguous; only precompute reads use strided/reversed APs.
The last layer runs forward-only (plus the single backward step that the
  final-timestep readout actually needs), then the 2-layer MLP head runs on
device. Output per core: (8, 50) = (out_dim, batch).
"""

import os
import sys

import numpy as np

_REPO = "/opt/trn_rl_repo"
if _REPO not in sys.path:
    sys.path.insert(0, _REPO)

B, KSEQ, T = 4, 100, 200
H = 128
L = 4
OUT = 8
NCORES = 8
N = B * KSEQ              # 400 sequences
NB = N // NCORES          # 50 per core
CT = 10                   # timesteps per precompute chunk
F16 = "float16"

_CACHE = {}


def _build_program(t_len=T, nb=NB, ct=CT):
    import concourse.bacc as bacc
    import concourse.mybir as mybir
    import concourse.tile as tile
    from contextlib import ExitStack

    f32 = mybir.dt.float32
    f16 = mybir.dt.float16

    nch = t_len // ct
    W = 2 * nb                  # 100: one x_cat block
    GIW = 6 * nb                # 300: one gi block [r_f r_b z_f z_b n_f n_b]

    nc = bacc.Bacc("TRN2", target_bir_lowering=False, debug=False,
                   num_devices=NCORES)

    # ---- DRAM I/O ----
    dx0f = nc.dram_tensor("x0f", (2, t_len * nb), f16, kind="ExternalInput").ap()
    dx0r = nc.dram_tensor("x0r", (2, t_len * nb), f16, kind="ExternalInput").ap()
    dw0 = nc.dram_tensor("w0", (2, 6 * H), f16, kind="ExternalInput").ap()
    dwih = nc.dram_tensor("wihT", (36, H, H), f16, kind="ExternalInput").ap()
    dwhh = nc.dram_tensor("whhT", (24, H, H), f16, kind="ExternalInput").ap()
    dbcols = nc.dram_tensor("bcols", (H, 18), f32, kind="ExternalInput").ap()
    dbhhn = nc.dram_tensor("bhhn", (H, 8), f32, kind="ExternalInput").ap()
    dident = nc.dram_tensor("ident", (H, H), f16, kind="ExternalInput").ap()
    dw1 = nc.dram_tensor("w1T", (2, H, H), f16, kind="ExternalInput").ap()
    db1 = nc.dram_tensor("b1col", (H, 1), f32, kind="ExternalInput").ap()
    dw2 = nc.dram_tensor("w2T", (H, OUT), f32, kind="ExternalInput").ap()
    db2 = nc.dram_tensor("b2col", (OUT, 1), f32, kind="ExternalInput").ap()
    dout = nc.dram_tensor("out", (OUT, nb), f32, kind="ExternalOutput").ap()

    with tile.TileContext(nc) as tc, ExitStack() as ctx:
        cpool = ctx.enter_context(tc.tile_pool(name="consts", bufs=1))
        xpool = ctx.enter_context(tc.tile_pool(name="xcat", bufs=1))
        gipool = ctx.enter_context(tc.tile_pool(name="gi", bufs=2))
        ppre = ctx.enter_context(tc.tile_pool(name="ppre", bufs=2, space="PSUM"))
        prz = ctx.enter_context(tc.tile_pool(name="prz", bufs=2, space="PSUM"))
        pq = ctx.enter_context(tc.tile_pool(name="pq", bufs=2, space="PSUM"))
        spool = ctx.enter_context(tc.tile_pool(name="scratch", bufs=3))
        hpool = ctx.enter_context(tc.tile_pool(name="hstate", bufs=3))

        # ---- constants / weights to SBUF ----
        w0_sb = cpool.tile([2, 6 * H], f16)
        nc.sync.dma_start(w0_sb[:], dw0)
        wih_sb = cpool.tile([H, 36 * H], f16)
        nc.sync.dma_start(wih_sb[:].rearrange("p (i c) -> p i c", c=H),
                          dwih.rearrange("i p c -> p i c"))
        whh_sb = cpool.tile([H, 24 * H], f16)
        nc.sync.dma_start(whh_sb[:].rearrange("p (i c) -> p i c", c=H),
                          dwhh.rearrange("i p c -> p i c"))
        bcols_sb = cpool.tile([H, 18], f32)
        nc.sync.dma_start(bcols_sb[:], dbcols)
        bhhn_sb = cpool.tile([H, 8], f32)
        nc.sync.dma_start(bhhn_sb[:], dbhhn)
        id_sb = cpool.tile([H, H], f16)
        nc.sync.dma_start(id_sb[:], dident)
        w1_sb = cpool.tile([H, 2 * H], f16)
        nc.sync.dma_start(w1_sb[:].rearrange("p (i c) -> p i c", c=H),
                          dw1.rearrange("i p c -> p i c"))
        b1_sb = cpool.tile([H, 1], f32)
        nc.sync.dma_start(b1_sb[:], db1)
        w2_sb = cpool.tile([H, OUT], f32)
        nc.sync.dma_start(w2_sb[:], dw2)
        b2_sb = cpool.tile([OUT, 1], f32)
        nc.sync.dma_start(b2_sb[:], db2)

        xA = xpool.tile([H, t_len * W], f16, tag="xA")
        xB = xpool.tile([H, t_len * W], f16, tag="xB")

        def wih_t(l, d, g, k):  # layers 1..3
            i = (((l - 1) * 2 + d) * 3 + g) * 2 + k
            return wih_sb[:, i * H:(i + 1) * H]

        def whh_t(l, d, g):
            i = (l * 2 + d) * 3 + g
            return whh_sb[:, i * H:(i + 1) * H]

        def bcol(l, d, g):
            return bcols_sb[:, (l - 1) * 6 + d * 3 + g:(l - 1) * 6 + d * 3 + g + 1]

        def bhhn_col(l, d):
            return bhhn_sb[:, l * 2 + d:l * 2 + d + 1]

        # ---------------- precompute ----------------
        def precompute_l0(x0f_sb, x0r_sb, c):
            """Layer-0 gi chunk c -> gi tile (ret). K=2 matmul incl bias row."""
            gi = gipool.tile([H, ct * GIW], f16, tag="gi")
            gi3 = gi[:].rearrange("p (t w) -> p t w", w=GIW)
            for d in range(2):
                src = x0f_sb if d == 0 else x0r_sb
                rhs = src[:, c * ct * nb:(c + 1) * ct * nb]
                for g in range(3):
                    ps = ppre.tile([H, ct * nb], f32, tag="ppre")
                    lhsT = w0_sb[:, (d * 3 + g) * H:(d * 3 + g + 1) * H]
                    nc.tensor.matmul(ps[:], lhsT, rhs, start=True, stop=True)
                    off = g * W + d * nb
                    nc.scalar.activation(
                        gi3[:, :, off:off + nb],
                        ps[:].rearrange("p (t n) -> p t n", n=nb),
                        mybir.ActivationFunctionType.Identity)
            return gi

        def precompute_l(l, x_in, c, dirs=(0, 1)):
            """Layers 1..3 gi chunk c. x_in blocks: [fwd@t | bwd@scanstep]."""
            gi = gipool.tile([H, ct * GIW], f16, tag="gi")
            gi3 = gi[:].rearrange("p (t w) -> p t w", w=GIW)
            x3 = x_in[:].rearrange("p (t w) -> p t w", w=W)
            s0 = c * ct
            hi = t_len - 1 - s0
            lo = hi - ct
            asc = slice(s0, s0 + ct)
            dsc = slice(hi, lo if lo >= 0 else None, -1)
            for d in dirs:
                # contract over prev fwd (k=0) then prev bwd (k=1)
                r0 = x3[:, asc if d == 0 else dsc, 0:nb]
                r1 = x3[:, dsc if d == 0 else asc, nb:W]
                for g in range(3):
                    ps = ppre.tile([H, ct * nb], f32, tag="ppre")
                    nc.tensor.matmul(ps[:], wih_t(l, d, g, 0), r0,
                                     start=True, stop=False)
                    nc.tensor.matmul(ps[:], wih_t(l, d, g, 1), r1,
                                     start=False, stop=True)
                    off = g * W + d * nb
                    nc.scalar.activation(
                        gi3[:, :, off:off + nb],
                        ps[:].rearrange("p (t n) -> p t n", n=nb),
                        mybir.ActivationFunctionType.Identity,
                        bias=bcol(l, d, g))
            return gi

        # ---------------- scan ----------------
        def scan_step(l, s, gi, tl, h_prev, x_out):
            """One both-direction GRU step. h_prev: (128, W) [f|b].
            Writes h' into x_out block s (layers 0-2) and returns the AP."""
            gi3 = gi[:].rearrange("p (t w) -> p t w", w=GIW)
            rz = prz.tile([H, 4 * nb], f32, tag="prz")
            q = pq.tile([H, W], f32, tag="pq")
            # psum prefill with gi[r|z] via identity matmul, then accumulate
            nc.tensor.matmul(rz[:], id_sb[:], gi3[:, tl, 0:4 * nb],
                             start=True, stop=False)
            for d in range(2):
                hd = h_prev[:, d * nb:(d + 1) * nb]
                nc.tensor.matmul(rz[:, d * nb:(d + 1) * nb],
                                 whh_t(l, d, 0), hd, start=False, stop=False)
                nc.tensor.matmul(rz[:, W + d * nb:W + (d + 1) * nb],
                                 whh_t(l, d, 1), hd, start=False, stop=(d == 1))
                nc.tensor.matmul(q[:, d * nb:(d + 1) * nb],
                                 whh_t(l, d, 2), hd,
                                 start=(d == 0), stop=(d == 1))
            rz_sb = spool.tile([H, 4 * nb], f16, tag="rz_sb")
            nc.scalar.activation(rz_sb[:], rz[:],
                                 mybir.ActivationFunctionType.Sigmoid)
            tmp = spool.tile([H, W], f16, tag="tmp")
            for d in range(2):
                sl = slice(d * nb, (d + 1) * nb)
                nc.vector.scalar_tensor_tensor(
                    tmp[:, sl], q[:, sl], bhhn_col(l, d), rz_sb[:, sl],
                    op0=mybir.AluOpType.add, op1=mybir.AluOpType.mult)
            n2 = spool.tile([H, W], f16, tag="n2")
            nc.vector.tensor_tensor(n2[:], tmp[:], gi3[:, tl, 4 * nb:GIW],
                                    op=mybir.AluOpType.add)
            n_sb = spool.tile([H, W], f16, tag="n_sb")
            nc.scalar.activation(n_sb[:], n2[:],
                                 mybir.ActivationFunctionType.Tanh)
            dd = spool.tile([H, W], f16, tag="dd")
            nc.vector.tensor_tensor(dd[:], h_prev, n_sb[:],
                                    op=mybir.AluOpType.subtract)
            zd = spool.tile([H, W], f16, tag="zd")
            nc.vector.tensor_tensor(zd[:], rz_sb[:, W:2 * W], dd[:],
                                    op=mybir.AluOpType.mult)
            if x_out is not None:
                h_new = x_out[:].rearrange("p (t w) -> p t w", w=W)[:, s, :]
            else:
                h_new = hpool.tile([H, W], f16, tag="h")[:]
            nc.vector.tensor_tensor(h_new, n_sb[:], zd[:],
                                    op=mybir.AluOpType.add)
            return h_new

        def scan_step_fwd(l, gi, tl, h_prev):
            """Forward-only GRU step for the last layer. h_prev: (128, nb)."""
            gi3 = gi[:].rearrange("p (t w) -> p t w", w=GIW)
            gi4 = gi[:].rearrange("p (t a n) -> p t a n", a=6, n=nb)
            rz = prz.tile([H, 2 * nb], f32, tag="prz")
            q = pq.tile([H, nb], f32, tag="pq")
            nc.tensor.matmul(rz[:], id_sb[:], gi4[:, tl, 0:4:2, :],
                             start=True, stop=False)
            nc.tensor.matmul(rz[:, 0:nb], whh_t(l, 0, 0), h_prev,
                             start=False, stop=False)
            nc.tensor.matmul(rz[:, nb:2 * nb], whh_t(l, 0, 1), h_prev,
                             start=False, stop=True)
            nc.tensor.matmul(q[:], whh_t(l, 0, 2), h_prev,
                             start=True, stop=True)
            rz_sb = spool.tile([H, 2 * nb], f16, tag="rzf_sb")
            nc.scalar.activation(rz_sb[:], rz[:],
                                 mybir.ActivationFunctionType.Sigmoid)
            tmp = spool.tile([H, nb], f16, tag="tmpf")
            nc.vector.scalar_tensor_tensor(
                tmp[:], q[:], bhhn_col(l, 0), rz_sb[:, 0:nb],
                op0=mybir.AluOpType.add, op1=mybir.AluOpType.mult)
            n2 = spool.tile([H, nb], f16, tag="n2f")
            nc.vector.tensor_tensor(n2[:], tmp[:], gi3[:, tl, 4 * nb:5 * nb],
                                    op=mybir.AluOpType.add)
            n_sb = spool.tile([H, nb], f16, tag="nf_sb")
            nc.scalar.activation(n_sb[:], n2[:],
                                 mybir.ActivationFunctionType.Tanh)
            dd = spool.tile([H, nb], f16, tag="ddf")
            nc.vector.tensor_tensor(dd[:], h_prev, n_sb[:],
                                    op=mybir.AluOpType.subtract)
            zd = spool.tile([H, nb], f16, tag="zdf")
            nc.vector.tensor_tensor(zd[:], rz_sb[:, nb:2 * nb], dd[:],
                                    op=mybir.AluOpType.mult)
            h_new = hpool.tile([H, nb], f16, tag="hf")
            nc.vector.tensor_tensor(h_new[:], n_sb[:], zd[:],
                                    op=mybir.AluOpType.add)
            return h_new

        # ---------------- layers 0..2 (full bidirectional) ----------------
        with tc.tile_pool(name="l0feed", bufs=1) as fpool:
            x0f_sb = fpool.tile([2, t_len * nb], f16)
            nc.sync.dma_start(x0f_sb[:], dx0f)
            x0r_sb = fpool.tile([2, t_len * nb], f16)
            nc.sync.dma_start(x0r_sb[:], dx0r)

            for l, x_in, x_out in ((0, None, xA), (1, xA, xB), (2, xB, xA)):
                h0 = hpool.tile([H, W], f16, tag="h")
                nc.vector.memset(h0[:], 0.0)
                h = h0[:]
                if l == 0:
                    pre = lambda c: precompute_l0(x0f_sb, x0r_sb, c)
                else:
                    pre = lambda c: precompute_l(l, x_in, c)
                gis = [pre(0), pre(1)]
                for c in range(nch):
                    gi = gis[c % 2]
                    for tl in range(ct):
                        h = scan_step(l, c * ct + tl, gi, tl, h, x_out)
                    if c + 2 < nch:
                        gis[c % 2] = pre(c + 2)

        # ---------------- layer 3: fwd scan + single bwd step -------------
        l = 3
        hf0 = hpool.tile([H, nb], f16, tag="hf")
        nc.vector.memset(hf0[:], 0.0)
        hf = hf0
        gis = [precompute_l(l, xA, 0, dirs=(0, 1)),
               precompute_l(l, xA, 1, dirs=(0,))]
        gi0 = gis[0]
        for c in range(nch):
            gi = gis[c % 2]
            for tl in range(ct):
                hf = scan_step_fwd(l, gi, tl, hf[:])
            if c + 2 < nch:
                gis[c % 2] = precompute_l(l, xA, c + 2, dirs=(0,))

        # backward single step (h0 = 0): uses gi chunk 0, tl = 0, bwd slices
        g03 = gi0[:].rearrange("p (t w) -> p t w", w=GIW)
        rb = spool.tile([H, nb], f16, tag="rb")
        nc.scalar.activation(rb[:], g03[:, 0, nb:2 * nb],
                             mybir.ActivationFunctionType.Sigmoid)
        zb = spool.tile([H, nb], f16, tag="zb")
        nc.scalar.activation(zb[:], g03[:, 0, W + nb:W + 2 * nb],
                             mybir.ActivationFunctionType.Sigmoid)
        nb2 = spool.tile([H, nb], f16, tag="nb2")
        nc.vector.scalar_tensor_tensor(
            nb2[:], rb[:], bhhn_col(l, 1), g03[:, 0, 5 * nb:6 * nb],
            op0=mybir.AluOpType.mult, op1=mybir.AluOpType.add)
        nbt = spool.tile([H, nb], f16, tag="nbt")
        nc.scalar.activation(nbt[:], nb2[:], mybir.ActivationFunctionType.Tanh)
        zn = spool.tile([H, nb], f16, tag="zn")
        nc.vector.tensor_tensor(zn[:], zb[:], nbt[:], op=mybir.AluOpType.mult)
        hb = hpool.tile([H, nb], f16, tag="hb")
        nc.vector.tensor_tensor(hb[:], nbt[:], zn[:],
                                op=mybir.AluOpType.subtract)

        # ---------------- MLP head ----------------
        with tc.tile_pool(name="phead", bufs=1, space="PSUM") as php:
            ph1 = php.tile([H, nb], f32)
            nc.tensor.matmul(ph1[:], w1_sb[:, 0:H], hf[:],
                             start=True, stop=False)
            nc.tensor.matmul(ph1[:], w1_sb[:, H:2 * H], hb[:],
                             start=False, stop=True)
            h1p = spool.tile([H, nb], f32, tag="h1p")
            nc.scalar.activation(h1p[:], ph1[:],
                                 mybir.ActivationFunctionType.Identity,
                                 bias=b1_sb[:])
            h1 = spool.tile([H, nb], f32, tag="h1")
            nc.vector.scalar_tensor_tensor(
                h1[:], h1p[:], 0.2, h1p[:],
                op0=mybir.AluOpType.mult, op1=mybir.AluOpType.max)
            po = php.tile([OUT, nb], f32)
            nc.tensor.matmul(po[:], w2_sb[:], h1[:], start=True, stop=True)
            o_sb = spool.tile([OUT, nb], f32, tag="o_sb")
            nc.scalar.activation(o_sb[:], po[:],
                                 mybir.ActivationFunctionType.Identity,
                                 bias=b2_sb[:])
            nc.sync.dma_start(dout, o_sb[:])

    nc.compile()
    return nc


def _prep_host(raw, Wih0, Wih, Whh, bih, bhh, W1, b1, W2, b2,
               t_len=T, nb=NB):
    """Host-side weight/layout prep. Returns (shared_inputs, per_core_feeds)."""
    f16 = np.float16
    Wih0 = np.asarray(Wih0, np.float32)
    Wih = np.asarray(Wih, np.float32)
    Whh = np.asarray(Whh, np.float32)
    bih = np.asarray(bih, np.float32)
    bhh = np.asarray(bhh, np.float32)

    # layer0 lhsT (2, 6*128): row0 weights, row1 combined bias
    w0 = np.zeros((2, 6 * H), np.float32)
    for d in range(2):
        for g in range(3):
            sl = slice(g * H, (g + 1) * H)
            w0[0, (d * 3 + g) * H:(d * 3 + g + 1) * H] = Wih0[d, sl, 0]
            bb = bih[0, d, sl] + (bhh[0, d, sl] if g < 2 else 0.0)
            w0[1, (d * 3 + g) * H:(d * 3 + g + 1) * H] = bb

    wihT = np.zeros((36, H, H), np.float32)
    for l in range(1, 4):
        for d in range(2):
            for g in range(3):
                for k in range(2):
                    i = (((l - 1) * 2 + d) * 3 + g) * 2 + k
                    wihT[i] = Wih[l - 1, d, g * H:(g + 1) * H,
                                  k * H:(k + 1) * H].T
    whhT = np.zeros((24, H, H), np.float32)
    for l in range(4):
        for d in range(2):
            for g in range(3):
                whhT[(l * 2 + d) * 3 + g] = Whh[l, d, g * H:(g + 1) * H, :].T

    bcols = np.zeros((H, 18), np.float32)
    for l in range(1, 4):
        for d in range(2):
            for g in range(3):
                sl = slice(g * H, (g + 1) * H)
                bb = bih[l, d, sl] + (bhh[l, d, sl] if g < 2 else 0.0)
                bcols[:, (l - 1) * 6 + d * 3 + g] = bb
    bhhn = np.zeros((H, 8), np.float32)
    for l in range(4):
        for d in range(2):
            bhhn[:, l * 2 + d] = bhh[l, d, 2 * H:3 * H]

    shared = {
        "w0": w0.astype(f16),
        "wihT": wihT.astype(f16),
        "whhT": whhT.astype(f16),
        "bcols": bcols,
        "bhhn": bhhn,
        "ident": np.eye(H, dtype=f16),
        "w1T": np.stack([np.asarray(W1, np.float32)[:, 0:H].T,
                         np.asarray(W1, np.float32)[:, H:2 * H].T]).astype(f16),
        "b1col": np.asarray(b1, np.float32).reshape(H, 1),
        "w2T": np.asarray(W2, np.float32).T.copy(),
        "b2col": np.asarray(b2, np.float32).reshape(OUT, 1),
    }

    x = np.asarray(raw, np.float32).reshape(N, t_len)
    feeds = []
    for c in range(NCORES):
        xs = x[c * nb:(c + 1) * nb]            # (nb, t)
        x0f = np.ones((2, t_len * nb), np.float32)
        x0f[0] = xs.T.reshape(-1)              # col t*nb+n
        x0r = np.ones((2, t_len * nb), np.float32)
        x0r[0] = xs.T[::-1].reshape(-1)        # col s*nb+n = x[n, t-1-s]
        feeds.append({"x0f": x0f.astype(f16), "x0r": x0r.astype(f16)})
    return shared, feeds


def kernel(raw, Wih0, Wih, Whh, bih, bhh, W1, b1, W2, b2):
    from concourse.bass_utils import run_bass_kernel_spmd

    if "prog" not in _CACHE:
        _CACHE["prog"] = _build_program()
    nc = _CACHE["prog"]

    shared, feeds = _prep_host(raw, Wih0, Wih, Whh, bih, bhh, W1, b1, W2, b2)
    in_maps = [dict(shared, **feeds[c]) for c in range(NCORES)]
    res = run_bass_kernel_spmd(nc, in_maps, list(range(NCORES)),
                               **_CACHE.get("run_kwargs", {}))
    _CACHE["last_results"] = res
    outs = [np.asarray(res.results[c]["out"], np.float32) for c in range(NCORES)]
    full = np.concatenate(outs, axis=1)        # (8, 400)
    return np.ascontiguousarray(full.T).reshape(B, KSEQ, OUT).astype(np.float32)



# revision 2
# speedup vs baseline: 1.1581x; 1.1581x over previous
"""Trainium2 Bass kernel for 4-layer bidirectional GRU (H=128, T=200) + MLP head.

Data-parallel: 400 sequences -> 50 per core on 8 cores. Layout: 128 partitions
= hidden unit, free dim = batch slots [fwd 50 | bwd 50].

Per scan step the critical chain is only:
  2 r-gate matmuls -> sigmoid(r) -> tensor_tensor_scan #1 (r*(q+bhn)+gin)
  -> tanh -> tensor_tensor_scan #2 (h' = (1-z)*n + z*h)
Everything else (z/n matmuls, sigmoid(1-z), z*h product, PSUM prefills via
identity matmuls, input-projection precompute, evictions) runs off-chain on
idle engine slots.

Key tricks:
  - gi layout per step: [r(100) | z(100) | bg(200)] where bg interleaves
    (bhh_n bias, gi_n) pairs so ONE identity matmul prefills the PSUM bank
    that the n-gate matmuls then accumulate into (stride-2 PSUM writes).
  - tensor_tensor_scan pairs: with d0=[0|r] interleaved, state resets every
    pair, computing a 3-operand FMA in one DVE op.
  - sigmoid(-x) gives (1-z) directly (scale=-1).
  - h' is written interleaved [n|h']; matmuls read h with stride-2 rhs APs.
"""

import sys

import numpy as np

_REPO = "/opt/trn_rl_repo"
if _REPO not in sys.path:
    sys.path.insert(0, _REPO)

B, KSEQ, T = 4, 100, 200
H = 128
L = 4
OUT = 8
NCORES = 8
N = B * KSEQ
NB = N // NCORES          # 50 per core
CT = 10                   # timesteps per precompute chunk
F16 = "float16"

_CACHE = {}


def _build_program(t_len=T, nb=NB, ct=CT, num_devices=NCORES):
    import concourse.bacc as bacc
    import concourse.mybir as mybir
    import concourse.tile as tile
    from contextlib import ExitStack

    f32 = mybir.dt.float32
    f16 = mybir.dt.float16
    AF = mybir.ActivationFunctionType
    ALU = mybir.AluOpType

    nch = t_len // ct
    W = 2 * nb                  # 100: one x block / one direction-pair width
    GW = 4 * nb * 2             # 400: one gi step block [r|z|bg]
    GW3 = 2 * nb * 2            # 200: layer-3 gi step block [r|z|bg]

    nc = bacc.Bacc("TRN2", target_bir_lowering=False, debug=False,
                   num_devices=num_devices)

    # ---- DRAM I/O ----
    dx0f = nc.dram_tensor("x0f", (2, t_len * nb), f16, kind="ExternalInput").ap()
    dx0r = nc.dram_tensor("x0r", (2, t_len * nb), f16, kind="ExternalInput").ap()
    dw0 = nc.dram_tensor("w0", (2, 6 * H), f16, kind="ExternalInput").ap()
    dwih = nc.dram_tensor("wihT", (36, H, H), f16, kind="ExternalInput").ap()
    dwhh = nc.dram_tensor("whhT", (24, H, H), f16, kind="ExternalInput").ap()
    dbcols = nc.dram_tensor("bcols", (H, 18), f32, kind="ExternalInput").ap()
    # bhn broadcast patterns: full bg regions (bhn at evens, 0 at odds; the
    # odd gin slots are overwritten by evictions afterwards)
    dbhn012 = nc.dram_tensor("bhn012", (3, H, ct * 2 * W), f16,
                             kind="ExternalInput").ap()
    dbhn3 = nc.dram_tensor("bhn3", (H, ct * 2 * nb), f16,
                           kind="ExternalInput").ap()
    dbhn3b = nc.dram_tensor("bhn3b", (H, 2 * nb), f16,
                            kind="ExternalInput").ap()
    dident = nc.dram_tensor("ident", (H, H), f16, kind="ExternalInput").ap()
    dw1 = nc.dram_tensor("w1T", (2, H, H), f16, kind="ExternalInput").ap()
    db1 = nc.dram_tensor("b1col", (H, 1), f32, kind="ExternalInput").ap()
    dw2 = nc.dram_tensor("w2T", (H, OUT), f32, kind="ExternalInput").ap()
    db2 = nc.dram_tensor("b2col", (OUT, 1), f32, kind="ExternalInput").ap()
    dout = nc.dram_tensor("out", (OUT, nb), f32, kind="ExternalOutput").ap()

    with tile.TileContext(nc) as tc, ExitStack() as ctx:
        cpool = ctx.enter_context(tc.tile_pool(name="consts", bufs=1))
        pers = ctx.enter_context(tc.tile_pool(name="pers", bufs=1))
        prz = ctx.enter_context(tc.tile_pool(name="prz", bufs=2, space="PSUM"))
        pqg = ctx.enter_context(tc.tile_pool(name="pqg", bufs=2, space="PSUM"))
        ppre = ctx.enter_context(tc.tile_pool(name="ppre", bufs=2, space="PSUM"))

        # ---- constants / weights ----
        w0_sb = cpool.tile([2, 6 * H], f16)
        nc.sync.dma_start(w0_sb[:], dw0)
        wih_sb = cpool.tile([H, 36 * H], f16)
        nc.sync.dma_start(wih_sb[:].rearrange("p (i c) -> p i c", c=H),
                          dwih.rearrange("i p c -> p i c"))
        whh_sb = cpool.tile([H, 24 * H], f16)
        nc.sync.dma_start(whh_sb[:].rearrange("p (i c) -> p i c", c=H),
                          dwhh.rearrange("i p c -> p i c"))
        bcols_sb = cpool.tile([H, 18], f32)
        nc.sync.dma_start(bcols_sb[:], dbcols)
        id_sb = cpool.tile([H, H], f16)
        nc.sync.dma_start(id_sb[:], dident)
        w1_sb = cpool.tile([H, 2 * H], f16)
        nc.sync.dma_start(w1_sb[:].rearrange("p (i c) -> p i c", c=H),
                          dw1.rearrange("i p c -> p i c"))
        b1_sb = cpool.tile([H, 1], f32)
        nc.sync.dma_start(b1_sb[:], db1)
        w2_sb = cpool.tile([H, OUT], f32)
        nc.sync.dma_start(w2_sb[:], dw2)
        b2_sb = cpool.tile([OUT, 1], f32)
        nc.sync.dma_start(b2_sb[:], db2)

        x0f_sb = pers.tile([2, t_len * nb], f16, tag="x0f")
        nc.sync.dma_start(x0f_sb[:], dx0f)
        x0r_sb = pers.tile([2, t_len * nb], f16, tag="x0r")
        nc.sync.dma_start(x0r_sb[:], dx0r)

        # persistent state tiles
        xA = pers.tile([H, t_len * W], f16, tag="xA")
        xB = pers.tile([H, t_len * W], f16, tag="xB")
        gis = [pers.tile([H, ct * GW], f16, tag=f"gi{i}", name=f"gi{i}")
               for i in range(3)]
        sr = pers.tile([H, 2 * W], f16, tag="sr")       # [0|r] pairs
        szm = pers.tile([H, 2 * W], f16, tag="szm")     # [0|1-z] pairs
        tn2 = pers.tile([H, 2 * W], f16, tag="tn2")     # scan1 out [.|n2]
        sd1 = [pers.tile([H, 2 * W], f16, tag=f"sd1{i}", name=f"sd1{i}")
               for i in range(2)]
        obuf = [pers.tile([H, 2 * W], f16, tag=f"o{i}", name=f"o{i}")
                for i in range(3)]
        ut = pers.tile([H, W], f16, tag="ut")
        gib = pers.tile([H, GW3], f16, tag="gib")       # layer3 bwd single step

        nc.vector.memset(sr[:], 0.0)
        nc.vector.memset(szm[:], 0.0)

        def wih_t(l, d, g, k):  # layers 1..3
            i = (((l - 1) * 2 + d) * 3 + g) * 2 + k
            return wih_sb[:, i * H:(i + 1) * H]

        def whh_t(l, d, g):
            i = (l * 2 + d) * 3 + g
            return whh_sb[:, i * H:(i + 1) * H]

        def bcol(l, d, g):
            i = (l - 1) * 6 + d * 3 + g
            return bcols_sb[:, i:i + 1]

        def odds(tile_ap, w):
            """Odd elements of an interleaved pair region: (H, w) stride 2."""
            return tile_ap.rearrange("p (j two) -> p two j", two=2)[:, 1, 0:w]

        # ------------- precompute pieces (emitted interleaved) -------------
        def pre_pieces_l0(c, gi):
            """Layer-0 gi for chunk c -> list of emission closures."""
            gi3 = gi[:, 0:ct * GW].rearrange("p (tl w) -> p tl w", w=GW)
            pieces = []
            for d in range(2):
                src = x0f_sb if d == 0 else x0r_sb
                rhs = src[:, c * ct * nb:(c + 1) * ct * nb]
                for g in range(3):
                    ps = ppre.tile([H, ct * nb], f32, tag="ppre", name="ppret")
                    lhsT = w0_sb[:, (d * 3 + g) * H:(d * 3 + g + 1) * H]

                    def mm(ps=ps, lhsT=lhsT, rhs=rhs):
                        nc.tensor.matmul(ps[:], lhsT, rhs, start=True, stop=True)

                    if g < 2:
                        dst = gi3[:, :, g * W + d * nb: g * W + (d + 1) * nb]
                    else:
                        dst = gi3[:, :, 2 * W:].rearrange(
                            "p tl (j two) -> p tl two j", two=2
                        )[:, :, 1, d * nb:(d + 1) * nb]

                    def ev(ps=ps, dst=dst):
                        nc.scalar.activation(
                            dst, ps[:].rearrange("p (tl n) -> p tl n", n=nb),
                            AF.Identity)

                    pieces.append(mm)
                    pieces.append(ev)
            return pieces

        def pre_pieces(l, x_in, c, gi, dirs=(0, 1)):
            """Layers 1..3 gi for chunk c. gi block: [r|z|bg] per step."""
            gw = GW if l < 3 else GW3
            gi3 = gi[:, 0:ct * gw].rearrange("p (tl w) -> p tl w", w=gw)
            x3 = x_in[:].rearrange("p (t w) -> p t w", w=W)
            s0 = c * ct
            hi = t_len - 1 - s0
            lo = hi - ct
            asc = slice(s0, s0 + ct)
            dsc = slice(hi, lo if lo >= 0 else None, -1)
            pieces = []
            for d in dirs:
                r0 = x3[:, asc if d == 0 else dsc, 0:nb]
                r1 = x3[:, dsc if d == 0 else asc, nb:W]
                for g in range(3):
                    ps = ppre.tile([H, ct * nb], f32, tag="ppre", name="ppret")

                    def mm0(ps=ps, l=l, d=d, g=g, r0=r0):
                        nc.tensor.matmul(ps[:], wih_t(l, d, g, 0), r0,
                                         start=True, stop=False)

                    def mm1(ps=ps, l=l, d=d, g=g, r1=r1):
                        nc.tensor.matmul(ps[:], wih_t(l, d, g, 1), r1,
                                         start=False, stop=True)

                    gslot = W if l < 3 else nb
                    if g < 2:
                        dst = gi3[:, :, g * gslot + d * nb:
                                  g * gslot + (d + 1) * nb]
                    else:
                        dst = gi3[:, :, 2 * gslot:].rearrange(
                            "p tl (j two) -> p tl two j", two=2
                        )[:, :, 1, d * nb:(d + 1) * nb]

                    def ev(ps=ps, dst=dst, l=l, d=d, g=g):
                        nc.scalar.activation(
                            dst, ps[:].rearrange("p (tl n) -> p tl n", n=nb),
                            AF.Identity, bias=bcol(l, d, g))

                    pieces.append(mm0)
                    pieces.append(mm1)
                    pieces.append(ev)
            return pieces

        # ---------------------- one scan step ----------------------------
        def scan_step(l, s, gi, tl, o_prev, o_cur, sd, x_out, w):
            """w = W for layers 0-2 (both dirs), nb for layer 3 (fwd only)."""
            gw = 4 * w
            gi3 = gi[:, 0:ct * gw].rearrange("p (tl g) -> p tl g", g=gw)
            p_rz = prz.tile([H, 2 * W], f32, tag="prz", name="przt")[:, 0:2 * w]
            p_qg = pqg.tile([H, 2 * W], f32, tag="pqg", name="pqgt")[:, 0:2 * w]
            h_prev = odds(o_prev[:, 0:2 * w], w)

            # PSUM prefills (identity matmuls; off-chain)
            nc.tensor.matmul(p_rz, id_sb[:], gi3[:, tl, 0:2 * w],
                             start=True, stop=False)
            nc.tensor.matmul(p_qg, id_sb[:], gi3[:, tl, 2 * w:4 * w],
                             start=True, stop=False)
            # recurrent matmuls; r gates first so sigma_r starts early
            ndir = 2 if w == W else 1
            for d in range(ndir):
                hd = h_prev[:, d * nb:(d + 1) * nb]
                nc.tensor.matmul(p_rz[:, d * nb:(d + 1) * nb],
                                 whh_t(l, d, 0), hd, start=False, stop=False)
            for d in range(ndir):
                hd = h_prev[:, d * nb:(d + 1) * nb]
                nc.tensor.matmul(p_rz[:, w + d * nb:w + (d + 1) * nb],
                                 whh_t(l, d, 1), hd, start=False,
                                 stop=(d == ndir - 1))
            qev = p_qg.rearrange("p (j two) -> p two j", two=2)
            for d in range(ndir):
                hd = h_prev[:, d * nb:(d + 1) * nb]
                nc.tensor.matmul(qev[:, 0, d * nb:(d + 1) * nb],
                                 whh_t(l, d, 2), hd, start=False,
                                 stop=(d == ndir - 1))

            # ACT: sigma_r (chain), sigma_zm (off-chain)
            nc.scalar.activation(odds(sr[:, 0:2 * w], w), p_rz[:, 0:w],
                                 AF.Sigmoid)
            nc.scalar.activation(odds(szm[:, 0:2 * w], w), p_rz[:, w:2 * w],
                                 AF.Sigmoid, scale=-1.0)

            # scan1: n2 = r*(q+bhn) + gin  (odd outputs)
            nc.vector.tensor_tensor_scan(
                tn2[:, 0:2 * w], sr[:, 0:2 * w], p_qg, 0.0,
                op0=ALU.mult, op1=ALU.add)

            # tanh -> n into sd evens
            sdv = sd[:, 0:2 * w].rearrange("p (j two) -> p two j", two=2)
            nc.scalar.activation(sdv[:, 0, :], odds(tn2[:, 0:2 * w], w),
                                 AF.Tanh)

            # off-chain: zh = h - (1-z)*h  -> sd odds
            nc.vector.tensor_tensor(ut[:, 0:w], odds(szm[:, 0:2 * w], w),
                                    h_prev, op=ALU.mult)
            nc.vector.tensor_tensor(sdv[:, 1, :], h_prev, ut[:, 0:w],
                                    op=ALU.subtract)

            # scan2: h' = (1-z)*n + zh  (odd outputs of o_cur)
            nc.vector.tensor_tensor_scan(
                o_cur[:, 0:2 * w], szm[:, 0:2 * w], sd[:, 0:2 * w], 0.0,
                op0=ALU.mult, op1=ALU.add)

            if x_out is not None:
                blk = x_out[:].rearrange("p (t w) -> p t w", w=W)[:, s, :]
                nc.vector.tensor_copy(blk, odds(o_cur[:], W))

        # ------------------- layer driver --------------------------------
        def run_layer(l, x_in, x_out, w, dirs=(0, 1), pre_extra=None):
            """Scan all t_len steps of layer l, interleaving precompute."""
            if l == 0:
                pre = lambda c, gi: pre_pieces_l0(c, gi)
            else:
                pre = lambda c, gi: pre_pieces(l, x_in, c, gi, dirs)
            # bhn broadcast into the full bg regions of all three gi buffers
            # (bhn at evens, zeros at odds; evictions then overwrite the odds)
            for i in range(3):
                gbg = gis[i][:, 0:ct * (4 * w)].rearrange(
                    "p (tl g) -> p tl g", g=4 * w)[:, :, 2 * w:]
                src = dbhn012[l].rearrange("p (tl j) -> p tl j", j=2 * W) \
                    if l < 3 else dbhn3.rearrange("p (tl j) -> p tl j", j=2 * nb)
                nc.sync.dma_start(gbg, src)
            # first two chunks up-front
            for piece in pre(0, gis[0]):
                piece()
            for piece in pre(1, gis[1]):
                piece()
            # zero the h buffer read at step 0
            nc.vector.memset(obuf[2][:], 0.0)
            queue = []
            for c in range(nch):
                gi = gis[c % 3]
                if c + 2 < nch:
                    queue = list(pre(c + 2, gis[(c + 2) % 3]))
                elif pre_extra is not None and c == nch - 1:
                    queue = list(pre_extra)
                    pre_extra = None
                k = max(1, (len(queue) + ct - 1) // ct) if queue else 0
                for tl in range(ct):
                    s = c * ct + tl
                    scan_step(l, s, gi, tl, obuf[(s + 2) % 3],
                              obuf[s % 3], sd1[s % 2], x_out, w)
                    for _ in range(k):
                        if queue:
                            queue.pop(0)()
            while queue:
                queue.pop(0)()
            return obuf[(t_len - 1) % 3]

        run_layer(0, None, xA, W)
        run_layer(1, xA, xB, W)
        run_layer(2, xB, xA, W)

        # layer 3: forward-only scan; bwd single-step precompute hooks in at
        # the end of the layer's own precompute stream
        x3v = xA[:].rearrange("p (t w) -> p t w", w=W)
        bwd_pieces = []
        ps_b = ppre.tile([H, ct * nb], f32, tag="ppre", name="psb")[:, 0:3 * nb]
        for g in range(3):
            def mm0(g=g):
                nc.tensor.matmul(ps_b[:, g * nb:(g + 1) * nb],
                                 wih_t(3, 1, g, 0), x3v[:, t_len - 1, 0:nb],
                                 start=True, stop=False)

            def mm1(g=g):
                nc.tensor.matmul(ps_b[:, g * nb:(g + 1) * nb],
                                 wih_t(3, 1, g, 1), x3v[:, 0, nb:W],
                                 start=False, stop=True)

            if g < 2:
                dst = gib[:, g * nb:(g + 1) * nb]
            else:
                dst = odds(gib[:, 2 * nb:4 * nb], nb)

            def ev(g=g, dst=dst):
                nc.scalar.activation(
                    dst, ps_b[:, g * nb:(g + 1) * nb], AF.Identity,
                    bias=bcol(3, 1, g))

            bwd_pieces += [mm0, mm1, ev]

        def bhn_b_dma():
            nc.sync.dma_start(gib[:, 2 * nb:4 * nb], dbhn3b)

        hf_o = run_layer(3, xA, None, nb, dirs=(0,),
                         pre_extra=[bhn_b_dma] + bwd_pieces)
        hf = odds(hf_o[:, 0:2 * nb], nb)

        # ---- layer-3 backward single step (h0 = 0) ----
        ob = obuf[t_len % 3]          # free buffer
        nc.vector.memset(ob[:], 0.0)  # h_prev = 0
        nc.scalar.activation(odds(sr[:, 0:2 * nb], nb), gib[:, 0:nb],
                             AF.Sigmoid)
        nc.scalar.activation(odds(szm[:, 0:2 * nb], nb), gib[:, nb:2 * nb],
                             AF.Sigmoid, scale=-1.0)
        nc.vector.tensor_tensor_scan(
            tn2[:, 0:2 * nb], sr[:, 0:2 * nb], gib[:, 2 * nb:4 * nb], 0.0,
            op0=ALU.mult, op1=ALU.add)
        sdv = sd1[0][:, 0:2 * nb].rearrange("p (j two) -> p two j", two=2)
        nc.scalar.activation(sdv[:, 0, :], odds(tn2[:, 0:2 * nb], nb), AF.Tanh)
        nc.vector.memset(sdv[:, 1, :], 0.0)
        nc.vector.tensor_tensor_scan(
            ob[:, 0:2 * nb], szm[:, 0:2 * nb], sd1[0][:, 0:2 * nb], 0.0,
            op0=ALU.mult, op1=ALU.add)
        hb = odds(ob[:, 0:2 * nb], nb)

        # ---------------- MLP head ----------------
        with tc.tile_pool(name="phead", bufs=1, space="PSUM") as php, \
                tc.tile_pool(name="shead", bufs=1) as shp:
            ph1 = php.tile([H, nb], f32)
            nc.tensor.matmul(ph1[:], w1_sb[:, 0:H], hf, start=True, stop=False)
            nc.tensor.matmul(ph1[:], w1_sb[:, H:2 * H], hb,
                             start=False, stop=True)
            h1p = shp.tile([H, nb], f32)
            nc.scalar.activation(h1p[:], ph1[:], AF.Identity, bias=b1_sb[:])
            h1 = shp.tile([H, nb], f32)
            nc.vector.scalar_tensor_tensor(
                h1[:], h1p[:], 0.2, h1p[:],
                op0=ALU.mult, op1=ALU.max)
            po = php.tile([OUT, nb], f32)
            nc.tensor.matmul(po[:], w2_sb[:], h1[:], start=True, stop=True)
            o_sb = shp.tile([OUT, nb], f32)
            nc.scalar.activation(o_sb[:], po[:], AF.Identity, bias=b2_sb[:])
            nc.sync.dma_start(dout, o_sb[:])

    nc.compile()
    return nc


def _prep_host(raw, Wih0, Wih, Whh, bih, bhh, W1, b1, W2, b2,
               t_len=T, nb=NB, ct=CT, ncores=NCORES):
    """Host-side weight/layout prep. Returns (shared_inputs, per_core_feeds)."""
    f16 = np.float16
    Wih0 = np.asarray(Wih0, np.float32)
    Wih = np.asarray(Wih, np.float32)
    Whh = np.asarray(Whh, np.float32)
    bih = np.asarray(bih, np.float32)
    bhh = np.asarray(bhh, np.float32)
    W = 2 * nb

    # layer0 lhsT (2, 6*128): row0 weights, row1 combined bias
    w0 = np.zeros((2, 6 * H), np.float32)
    for d in range(2):
        for g in range(3):
            sl = slice(g * H, (g + 1) * H)
            w0[0, (d * 3 + g) * H:(d * 3 + g + 1) * H] = Wih0[d, sl, 0]
            bb = bih[0, d, sl] + (bhh[0, d, sl] if g < 2 else 0.0)
            w0[1, (d * 3 + g) * H:(d * 3 + g + 1) * H] = bb

    wihT = np.zeros((36, H, H), np.float32)
    for l in range(1, 4):
        for d in range(2):
            for g in range(3):
                for k in range(2):
                    i = (((l - 1) * 2 + d) * 3 + g) * 2 + k
                    wihT[i] = Wih[l - 1, d, g * H:(g + 1) * H,
                                  k * H:(k + 1) * H].T
    whhT = np.zeros((24, H, H), np.float32)
    for l in range(4):
        for d in range(2):
            for g in range(3):
                whhT[(l * 2 + d) * 3 + g] = Whh[l, d, g * H:(g + 1) * H, :].T

    bcols = np.zeros((H, 18), np.float32)
    for l in range(1, 4):
        for d in range(2):
            for g in range(3):
                sl = slice(g * H, (g + 1) * H)
                bb = bih[l, d, sl] + (bhh[l, d, sl] if g < 2 else 0.0)
                bcols[:, (l - 1) * 6 + d * 3 + g] = bb

    # bhn (= bhh n-gate) patterns covering full bg regions: evens = bhn
    # broadcast, odds = 0 (gin slots, rewritten by evictions)
    def _bg(l, dirs):
        pat = np.zeros((H, 2 * nb * len(dirs)), np.float32)
        for j, d in enumerate(dirs):
            col = bhh[l, d, 2 * H:3 * H]
            pat[:, 2 * nb * j:2 * nb * (j + 1):2] = np.repeat(
                col[:, None], nb, 1)
        return pat

    bhn012 = np.zeros((3, H, ct * 2 * W), np.float32)
    for l in range(3):
        bhn012[l] = np.tile(_bg(l, (0, 1)), (1, ct))
    bhn3 = np.tile(_bg(3, (0,)), (1, ct))
    bhn3b = _bg(3, (1,))

    shared = {
        "w0": w0.astype(f16),
        "wihT": wihT.astype(f16),
        "whhT": whhT.astype(f16),
        "bcols": bcols,
        "bhn012": bhn012.astype(f16),
        "bhn3": bhn3.astype(f16),
        "bhn3b": bhn3b.astype(f16),
        "ident": np.eye(H, dtype=f16),
        "w1T": np.stack([np.asarray(W1, np.float32)[:, 0:H].T,
                         np.asarray(W1, np.float32)[:, H:2 * H].T]).astype(f16),
        "b1col": np.asarray(b1, np.float32).reshape(H, 1),
        "w2T": np.asarray(W2, np.float32).T.copy(),
        "b2col": np.asarray(b2, np.float32).reshape(OUT, 1),
    }

    x = np.asarray(raw, np.float32).reshape(-1, t_len)
    feeds = []
    for c in range(ncores):
        xs = x[c * nb:(c + 1) * nb]            # (nb, t)
        x0f = np.ones((2, t_len * nb), np.float32)
        x0f[0] = xs.T.reshape(-1)
        x0r = np.ones((2, t_len * nb), np.float32)
        x0r[0] = xs.T[::-1].reshape(-1)
        feeds.append({"x0f": x0f.astype(f16), "x0r": x0r.astype(f16)})
    return shared, feeds


def kernel(raw, Wih0, Wih, Whh, bih, bhh, W1, b1, W2, b2):
    from concourse.bass_utils import run_bass_kernel_spmd

    if "prog" not in _CACHE:
        _CACHE["prog"] = _build_program()
    nc = _CACHE["prog"]

    shared, feeds = _prep_host(raw, Wih0, Wih, Whh, bih, bhh, W1, b1, W2, b2)
    in_maps = [dict(shared, **feeds[c]) for c in range(NCORES)]
    res = run_bass_kernel_spmd(nc, in_maps, list(range(NCORES)),
                               **_CACHE.get("run_kwargs", {}))
    _CACHE["last_results"] = res
    outs = [np.asarray(res.results[c]["out"], np.float32) for c in range(NCORES)]
    full = np.concatenate(outs, axis=1)        # (8, 400)
    return np.ascontiguousarray(full.T).reshape(B, KSEQ, OUT).astype(np.float32)


# revision 3
# speedup vs baseline: 1.2456x; 1.0756x over previous
"""Trainium2 Bass kernel for 4-layer bidirectional GRU (H=128, T=200) + MLP head.

Data-parallel: 400 sequences -> 50 per core on 8 cores. Layout: 128 partitions
= hidden unit, free dim = batch slots [fwd 50 | bwd 50].

Per scan step the critical chain is:
  2 r-gate matmuls -> sigmoid(r) -> mul(r, q+bhn) -> add(gin) -> tanh
  -> mul(n, 1-z) -> add(z*h) -> h'
Off-chain: z/n matmuls, sigmoid(-z_pre) giving (1-z) directly, z*h product,
PSUM prefills via identity matmuls (biases + gi pre-loaded into the
accumulators), input-projection precompute and its PSUM->SBUF evictions
(split in halves to fit scheduling gaps), and a tiny PE-warming matmul after
each tanh to keep the PE out of its low-power state ahead of the gate
matmuls.

gi layout per step: [r(100) | z(100) | bhn(100) | gin(100)] - r/z/gin hold
input projections incl. biases (folded in the eviction's activation bias);
the bhn columns are DMA-broadcast once per layer so a single identity matmul
prefills the n-gate PSUM group with bhh_n before Whn*h accumulates onto it.
One (128,300) PSUM tile carries three independent accumulation groups
[r|z|q], so sigmoid(r) fires as soon as the two r matmuls stop.
"""

import sys

import numpy as np

_REPO = "/opt/trn_rl_repo"
if _REPO not in sys.path:
    sys.path.insert(0, _REPO)

B, KSEQ, T = 4, 100, 200
H = 128
L = 4
OUT = 8
NCORES = 8
N = B * KSEQ
NB = N // NCORES          # 50 per core
CT = 10                   # timesteps per precompute chunk
F16 = "float16"

_CACHE = {}


def _build_program(t_len=T, nb=NB, ct=CT, num_devices=NCORES):
    import concourse.bacc as bacc
    import concourse.mybir as mybir
    import concourse.tile as tile
    from contextlib import ExitStack

    f32 = mybir.dt.float32
    f16 = mybir.dt.float16
    AF = mybir.ActivationFunctionType
    ALU = mybir.AluOpType

    nch = t_len // ct
    W = 2 * nb                  # 100
    GW = 8 * nb                 # 400: gi step block [r|z|bhn|gin]
    GW3 = 4 * nb                # 200: layer-3 gi step block

    nc = bacc.Bacc("TRN2", target_bir_lowering=False, debug=False,
                   num_devices=num_devices)

    # ---- DRAM I/O ----
    dx0f = nc.dram_tensor("x0f", (2, t_len * nb), f16, kind="ExternalInput").ap()
    dx0r = nc.dram_tensor("x0r", (2, t_len * nb), f16, kind="ExternalInput").ap()
    dw0 = nc.dram_tensor("w0", (2, 6 * H), f16, kind="ExternalInput").ap()
    dwih = nc.dram_tensor("wihT", (36, H, H), f16, kind="ExternalInput").ap()
    dwhh = nc.dram_tensor("whhT", (24, H, H), f16, kind="ExternalInput").ap()
    dbcols = nc.dram_tensor("bcols", (H, 18), f32, kind="ExternalInput").ap()
    dbhn012 = nc.dram_tensor("bhn012", (3, H, ct * W), f16,
                             kind="ExternalInput").ap()
    dbhn3 = nc.dram_tensor("bhn3", (H, ct * nb), f16,
                           kind="ExternalInput").ap()
    dbhn3b = nc.dram_tensor("bhn3b", (H, nb), f16, kind="ExternalInput").ap()
    dident = nc.dram_tensor("ident", (H, H), f16, kind="ExternalInput").ap()
    dw1 = nc.dram_tensor("w1T", (2, H, H), f16, kind="ExternalInput").ap()
    db1 = nc.dram_tensor("b1col", (H, 1), f32, kind="ExternalInput").ap()
    dw2 = nc.dram_tensor("w2T", (H, OUT), f32, kind="ExternalInput").ap()
    db2 = nc.dram_tensor("b2col", (OUT, 1), f32, kind="ExternalInput").ap()
    dout = nc.dram_tensor("out", (OUT, nb), f32, kind="ExternalOutput").ap()

    with tile.TileContext(nc) as tc, ExitStack() as ctx:
        cpool = ctx.enter_context(tc.tile_pool(name="consts", bufs=1))
        pers = ctx.enter_context(tc.tile_pool(name="pers", bufs=1))
        pgate = ctx.enter_context(tc.tile_pool(name="pgate", bufs=1,
                                               space="PSUM"))
        ppre = ctx.enter_context(tc.tile_pool(name="ppre", bufs=2, space="PSUM"))
        pwarm = ctx.enter_context(tc.tile_pool(name="pwarm", bufs=1,
                                               space="PSUM"))

        # ---- constants / weights ----
        w0_sb = cpool.tile([2, 6 * H], f16)
        nc.sync.dma_start(w0_sb[:], dw0)
        wih_sb = cpool.tile([H, 36 * H], f16)
        nc.sync.dma_start(wih_sb[:].rearrange("p (i c) -> p i c", c=H),
                          dwih.rearrange("i p c -> p i c"))
        whh_sb = cpool.tile([H, 24 * H], f16)
        nc.sync.dma_start(whh_sb[:].rearrange("p (i c) -> p i c", c=H),
                          dwhh.rearrange("i p c -> p i c"))
        bcols_sb = cpool.tile([H, 18], f32)
        nc.sync.dma_start(bcols_sb[:], dbcols)
        id_sb = cpool.tile([H, H], f16)
        nc.sync.dma_start(id_sb[:], dident)
        w1_sb = cpool.tile([H, 2 * H], f16)
        nc.sync.dma_start(w1_sb[:].rearrange("p (i c) -> p i c", c=H),
                          dw1.rearrange("i p c -> p i c"))
        b1_sb = cpool.tile([H, 1], f32)
        nc.sync.dma_start(b1_sb[:], db1)
        w2_sb = cpool.tile([H, OUT], f32)
        nc.sync.dma_start(w2_sb[:], dw2)
        b2_sb = cpool.tile([OUT, 1], f32)
        nc.sync.dma_start(b2_sb[:], db2)

        x0f_sb = pers.tile([2, t_len * nb], f16, tag="x0f")
        nc.sync.dma_start(x0f_sb[:], dx0f)
        x0r_sb = pers.tile([2, t_len * nb], f16, tag="x0r")
        nc.sync.dma_start(x0r_sb[:], dx0r)

        # persistent state tiles
        xA = pers.tile([H, t_len * W], f16, tag="xA")
        xB = pers.tile([H, t_len * W], f16, tag="xB")
        gis = [pers.tile([H, ct * GW], f16, tag=f"gi{i}", name=f"gi{i}")
               for i in range(3)]
        r_sb = pers.tile([H, W], f16, tag="r_sb")
        zm_sb = pers.tile([H, W], f16, tag="zm_sb")
        tmp_sb = pers.tile([H, W], f16, tag="tmp_sb")
        n2_sb = pers.tile([H, W], f16, tag="n2_sb")
        n_sb = pers.tile([H, W], f16, tag="n_sb")
        u_sb = pers.tile([H, W], f16, tag="u_sb")
        zh_sb = pers.tile([H, W], f16, tag="zh_sb")
        nzm_sb = pers.tile([H, W], f16, tag="nzm_sb")
        zeros = pers.tile([H, W], f16, tag="zeros")
        hrot = [pers.tile([H, nb], f16, tag=f"hrot{i}", name=f"hrot{i}")
                for i in range(2)]
        hb_sb = pers.tile([H, nb], f16, tag="hb_sb")
        gib = pers.tile([H, GW3], f16, tag="gib")

        nc.vector.memset(zeros[:], 0.0)

        def wih_t(l, d, g, k):  # layers 1..3
            i = (((l - 1) * 2 + d) * 3 + g) * 2 + k
            return wih_sb[:, i * H:(i + 1) * H]

        def whh_t(l, d, g):
            i = (l * 2 + d) * 3 + g
            return whh_sb[:, i * H:(i + 1) * H]

        def bcol(l, d, g):
            i = (l - 1) * 6 + d * 3 + g
            return bcols_sb[:, i:i + 1]

        # ------------- precompute pieces (emitted interleaved) -------------
        def ev_halves(ps, dst3, bias):
            """Split one eviction into two halves along the chunk dim."""
            hh = ct // 2
            out = []
            for a in range(2):
                def ev(ps=ps, dst3=dst3, bias=bias, a=a):
                    src = ps[:].rearrange("p (tl n) -> p tl n", n=nb)
                    kw = {} if bias is None else {"bias": bias}
                    nc.scalar.activation(dst3[:, a * hh:(a + 1) * hh],
                                         src[:, a * hh:(a + 1) * hh],
                                         AF.Identity, **kw)
                out.append(ev)
            return out

        def pre_pieces_l0(c, gi):
            gi3 = gi[:, 0:ct * GW].rearrange("p (tl w) -> p tl w", w=GW)
            pieces = []
            for d in range(2):
                src = x0f_sb if d == 0 else x0r_sb
                rhs = src[:, c * ct * nb:(c + 1) * ct * nb]
                for g in range(3):
                    ps = ppre.tile([H, ct * nb], f32, tag="ppre", name="ppret")
                    lhsT = w0_sb[:, (d * 3 + g) * H:(d * 3 + g + 1) * H]

                    def mm(ps=ps, lhsT=lhsT, rhs=rhs):
                        nc.tensor.matmul(ps[:], lhsT, rhs, start=True,
                                         stop=True)

                    slot = g * W if g < 2 else 3 * W
                    dst3 = gi3[:, :, slot + d * nb: slot + (d + 1) * nb]
                    pieces.append(mm)
                    pieces += ev_halves(ps, dst3, None)
            return pieces

        def pre_pieces(l, x_in, c, gi, dirs=(0, 1)):
            gw = GW if l < 3 else GW3
            gslot = W if l < 3 else nb
            gi3 = gi[:, 0:ct * gw].rearrange("p (tl w) -> p tl w", w=gw)
            x3 = x_in[:].rearrange("p (t w) -> p t w", w=W)
            s0 = c * ct
            hi = t_len - 1 - s0
            lo = hi - ct
            asc = slice(s0, s0 + ct)
            dsc = slice(hi, lo if lo >= 0 else None, -1)
            pieces = []
            for d in dirs:
                r0 = x3[:, asc if d == 0 else dsc, 0:nb]
                r1 = x3[:, dsc if d == 0 else asc, nb:W]
                for g in range(3):
                    ps = ppre.tile([H, ct * nb], f32, tag="ppre", name="ppret")

                    def mm0(ps=ps, l=l, d=d, g=g, r0=r0):
                        nc.tensor.matmul(ps[:], wih_t(l, d, g, 0), r0,
                                         start=True, stop=False)

                    def mm1(ps=ps, l=l, d=d, g=g, r1=r1):
                        nc.tensor.matmul(ps[:], wih_t(l, d, g, 1), r1,
                                         start=False, stop=True)

                    slot = g * gslot if g < 2 else 3 * gslot
                    dst3 = gi3[:, :, slot + d * nb: slot + (d + 1) * nb]
                    pieces.append(mm0)
                    pieces.append(mm1)
                    pieces += ev_halves(ps, dst3, bcol(l, d, g))
            return pieces

        # ---------------------- one scan step ----------------------------
        def scan_step(l, s, gi, tl, h_prev, h_out, w):
            """h_prev/h_out: (H, w) APs. w = W for layers 0-2, nb for layer 3."""
            gw = 4 * w
            gi3 = gi[:, 0:ct * gw].rearrange("p (tl g) -> p tl g", g=gw)
            # one PSUM bank per accumulation group so all three can be open
            # at once (zero-out regions are bank-granular)
            P_r = pgate.tile([H, 512], f32, tag="p_r", name="p_r")[:, 0:w]
            P_z = pgate.tile([H, 512], f32, tag="p_z", name="p_z")[:, 0:w]
            P_q = pgate.tile([H, 512], f32, tag="p_q", name="p_q")[:, 0:w]

            nc.tensor.matmul(P_r, id_sb[:], gi3[:, tl, 0:w],
                             start=True, stop=False)
            nc.tensor.matmul(P_z, id_sb[:], gi3[:, tl, w:2 * w],
                             start=True, stop=False)
            nc.tensor.matmul(P_q, id_sb[:], gi3[:, tl, 2 * w:3 * w],
                             start=True, stop=False)
            ndir = 2 if w == W else 1
            for g, Pg in enumerate((P_r, P_z, P_q)):
                for d in range(ndir):
                    hd = h_prev[:, d * nb:(d + 1) * nb]
                    nc.tensor.matmul(Pg[:, d * nb:(d + 1) * nb],
                                     whh_t(l, d, g), hd, start=False,
                                     stop=(d == ndir - 1))

            # ACT: sigma_r (chain) then sigma_zm = sigmoid(-z_pre) (off-chain)
            nc.scalar.activation(r_sb[:, 0:w], P_r, AF.Sigmoid)
            nc.scalar.activation(zm_sb[:, 0:w], P_z, AF.Sigmoid,
                                 scale=-1.0)

            # DVE: tmp = r*(q+bhn); n2 = tmp + gin
            nc.vector.tensor_tensor(tmp_sb[:, 0:w], r_sb[:, 0:w],
                                    P_q, op=ALU.mult)
            nc.vector.tensor_tensor(n2_sb[:, 0:w], tmp_sb[:, 0:w],
                                    gi3[:, tl, 3 * w:4 * w], op=ALU.add)

            # ACT: n = tanh(n2)
            nc.scalar.activation(n_sb[:, 0:w], n2_sb[:, 0:w], AF.Tanh)

            # DVE (during tanh): u = zm*h; zh = h - u
            nc.vector.tensor_tensor(u_sb[:, 0:w], zm_sb[:, 0:w], h_prev,
                                    op=ALU.mult)
            nc.vector.tensor_tensor(zh_sb[:, 0:w], h_prev, u_sb[:, 0:w],
                                    op=ALU.subtract)

            # DVE: h' = n*zm + zh
            nc.vector.tensor_tensor(nzm_sb[:, 0:w], n_sb[:, 0:w],
                                    zm_sb[:, 0:w], op=ALU.mult)
            nc.vector.tensor_tensor(h_out, nzm_sb[:, 0:w], zh_sb[:, 0:w],
                                    op=ALU.add)

            # tiny matmul keyed on tanh output keeps the PE awake just before
            # the next step's gate matmuls
            pw = pwarm.tile([8, 8], f32, tag="pwarm", name="pwarmt")
            nc.tensor.matmul(pw[:], id_sb[:, 0:8], n_sb[:, 0:8],
                             start=True, stop=True)

        # ------------------- layer driver --------------------------------
        def run_layer(l, x_in, x_out, w, dirs=(0, 1), pre_extra=None):
            if l == 0:
                pre = lambda c, gi: pre_pieces_l0(c, gi)
            else:
                pre = lambda c, gi: pre_pieces(l, x_in, c, gi, dirs)
            gslot = W if l < 3 else nb
            for i in range(3):
                gbg = gis[i][:, 0:ct * 4 * gslot].rearrange(
                    "p (tl g) -> p tl g",
                    g=4 * gslot)[:, :, 2 * gslot:3 * gslot]
                src = dbhn012[l].rearrange("p (tl j) -> p tl j", j=W) \
                    if l < 3 else dbhn3.rearrange("p (tl j) -> p tl j", j=nb)
                nc.sync.dma_start(gbg, src)
            for piece in pre(0, gis[0]):
                piece()
            for piece in pre(1, gis[1]):
                piece()
            xo3 = None
            if x_out is not None:
                xo3 = x_out[:].rearrange("p (t w) -> p t w", w=W)
            queue = []
            for c in range(nch):
                gi = gis[c % 3]
                if c + 2 < nch:
                    queue = list(pre(c + 2, gis[(c + 2) % 3]))
                elif pre_extra is not None and c == nch - 1:
                    queue = list(pre_extra)
                    pre_extra = None
                k = max(1, (len(queue) + ct - 1) // ct) if queue else 0
                for tl in range(ct):
                    s = c * ct + tl
                    if l < 3:
                        h_prev = zeros[:, 0:w] if s == 0 else xo3[:, s - 1, :]
                        h_out = xo3[:, s, :]
                    else:
                        h_prev = zeros[:, 0:w] if s == 0 else \
                            hrot[(s - 1) % 2][:]
                        h_out = hrot[s % 2][:]
                    scan_step(l, s, gi, tl, h_prev, h_out, w)
                    for _ in range(k):
                        if queue:
                            queue.pop(0)()
            while queue:
                queue.pop(0)()

        run_layer(0, None, xA, W)
        run_layer(1, xA, xB, W)
        run_layer(2, xB, xA, W)

        # layer 3 fwd-only; its precompute tail also builds the single
        # backward-step gi (gib) from xA
        x3v = xA[:].rearrange("p (t w) -> p t w", w=W)
        bwd_pieces = []
        ps_b = ppre.tile([H, ct * nb], f32, tag="ppre", name="psb")[:, 0:3 * nb]
        for g in range(3):
            def mm0(g=g):
                nc.tensor.matmul(ps_b[:, g * nb:(g + 1) * nb],
                                 wih_t(3, 1, g, 0), x3v[:, t_len - 1, 0:nb],
                                 start=True, stop=False)

            def mm1(g=g):
                nc.tensor.matmul(ps_b[:, g * nb:(g + 1) * nb],
                                 wih_t(3, 1, g, 1), x3v[:, 0, nb:W],
                                 start=False, stop=True)

            slot = g * nb if g < 2 else 3 * nb

            def ev(g=g, slot=slot):
                nc.scalar.activation(gib[:, slot:slot + nb],
                                     ps_b[:, g * nb:(g + 1) * nb],
                                     AF.Identity, bias=bcol(3, 1, g))

            bwd_pieces += [mm0, mm1, ev]

        def bhn_b_dma():
            nc.sync.dma_start(gib[:, 2 * nb:3 * nb], dbhn3b)

        run_layer(3, xA, None, nb, dirs=(0,),
                  pre_extra=[bhn_b_dma] + bwd_pieces)
        hf = hrot[(t_len - 1) % 2][:]

        # ---- layer-3 backward single step (h0 = 0) ----
        nc.scalar.activation(r_sb[:, 0:nb], gib[:, 0:nb], AF.Sigmoid)
        nc.scalar.activation(zm_sb[:, 0:nb], gib[:, nb:2 * nb], AF.Sigmoid,
                             scale=-1.0)
        nc.vector.tensor_tensor(tmp_sb[:, 0:nb], r_sb[:, 0:nb],
                                gib[:, 2 * nb:3 * nb], op=ALU.mult)
        nc.vector.tensor_tensor(n2_sb[:, 0:nb], tmp_sb[:, 0:nb],
                                gib[:, 3 * nb:4 * nb], op=ALU.add)
        nc.scalar.activation(n_sb[:, 0:nb], n2_sb[:, 0:nb], AF.Tanh)
        nc.vector.tensor_tensor(hb_sb[:], n_sb[:, 0:nb], zm_sb[:, 0:nb],
                                op=ALU.mult)

        # ---------------- MLP head ----------------
        with tc.tile_pool(name="phead", bufs=1, space="PSUM") as php, \
                tc.tile_pool(name="shead", bufs=1) as shp:
            ph1 = php.tile([H, nb], f32)
            nc.tensor.matmul(ph1[:], w1_sb[:, 0:H], hf, start=True, stop=False)
            nc.tensor.matmul(ph1[:], w1_sb[:, H:2 * H], hb_sb[:],
                             start=False, stop=True)
            h1p = shp.tile([H, nb], f32)
            nc.scalar.activation(h1p[:], ph1[:], AF.Identity, bias=b1_sb[:])
            h1 = shp.tile([H, nb], f32)
            nc.vector.scalar_tensor_tensor(
                h1[:], h1p[:], 0.2, h1p[:],
                op0=ALU.mult, op1=ALU.max)
            po = php.tile([OUT, nb], f32)
            nc.tensor.matmul(po[:], w2_sb[:], h1[:], start=True, stop=True)
            o_sb = shp.tile([OUT, nb], f32)
            nc.scalar.activation(o_sb[:], po[:], AF.Identity, bias=b2_sb[:])
            nc.sync.dma_start(dout, o_sb[:])

    nc.compile()
    return nc


def _prep_host(raw, Wih0, Wih, Whh, bih, bhh, W1, b1, W2, b2,
               t_len=T, nb=NB, ct=CT, ncores=NCORES):
    """Host-side weight/layout prep. Returns (shared_inputs, per_core_feeds)."""
    f16 = np.float16
    Wih0 = np.asarray(Wih0, np.float32)
    Wih = np.asarray(Wih, np.float32)
    Whh = np.asarray(Whh, np.float32)
    bih = np.asarray(bih, np.float32)
    bhh = np.asarray(bhh, np.float32)
    W = 2 * nb

    w0 = np.zeros((2, 6 * H), np.float32)
    for d in range(2):
        for g in range(3):
            sl = slice(g * H, (g + 1) * H)
            w0[0, (d * 3 + g) * H:(d * 3 + g + 1) * H] = Wih0[d, sl, 0]
            bb = bih[0, d, sl] + (bhh[0, d, sl] if g < 2 else 0.0)
            w0[1, (d * 3 + g) * H:(d * 3 + g + 1) * H] = bb

    wihT = np.zeros((36, H, H), np.float32)
    for l in range(1, 4):
        for d in range(2):
            for g in range(3):
                for k in range(2):
                    i = (((l - 1) * 2 + d) * 3 + g) * 2 + k
                    wihT[i] = Wih[l - 1, d, g * H:(g + 1) * H,
                                  k * H:(k + 1) * H].T
    whhT = np.zeros((24, H, H), np.float32)
    for l in range(4):
        for d in range(2):
            for g in range(3):
                whhT[(l * 2 + d) * 3 + g] = Whh[l, d, g * H:(g + 1) * H, :].T

    bcols = np.zeros((H, 18), np.float32)
    for l in range(1, 4):
        for d in range(2):
            for g in range(3):
                sl = slice(g * H, (g + 1) * H)
                bb = bih[l, d, sl] + (bhh[l, d, sl] if g < 2 else 0.0)
                bcols[:, (l - 1) * 6 + d * 3 + g] = bb

    # bhn (= bhh n-gate) broadcast columns for the gi bhn slots
    def _bhncols(l, dirs):
        return np.concatenate([
            np.repeat(bhh[l, d, 2 * H:3 * H][:, None], nb, 1) for d in dirs],
            axis=1)

    bhn012 = np.zeros((3, H, ct * W), np.float32)
    for l in range(3):
        bhn012[l] = np.tile(_bhncols(l, (0, 1)), (1, ct))
    bhn3 = np.tile(_bhncols(3, (0,)), (1, ct))
    bhn3b = _bhncols(3, (1,))

    shared = {
        "w0": w0.astype(f16),
        "wihT": wihT.astype(f16),
        "whhT": whhT.astype(f16),
        "bcols": bcols,
        "bhn012": bhn012.astype(f16),
        "bhn3": bhn3.astype(f16),
        "bhn3b": bhn3b.astype(f16),
        "ident": np.eye(H, dtype=f16),
        "w1T": np.stack([np.asarray(W1, np.float32)[:, 0:H].T,
                         np.asarray(W1, np.float32)[:, H:2 * H].T]).astype(f16),
        "b1col": np.asarray(b1, np.float32).reshape(H, 1),
        "w2T": np.asarray(W2, np.float32).T.copy(),
        "b2col": np.asarray(b2, np.float32).reshape(OUT, 1),
    }

    x = np.asarray(raw, np.float32).reshape(-1, t_len)
    feeds = []
    for c in range(ncores):
        xs = x[c * nb:(c + 1) * nb]
        x0f = np.ones((2, t_len * nb), np.float32)
        x0f[0] = xs.T.reshape(-1)
        x0r = np.ones((2, t_len * nb), np.float32)
        x0r[0] = xs.T[::-1].reshape(-1)
        feeds.append({"x0f": x0f.astype(f16), "x0r": x0r.astype(f16)})
    return shared, feeds


def kernel(raw, Wih0, Wih, Whh, bih, bhh, W1, b1, W2, b2):
    from concourse.bass_utils import run_bass_kernel_spmd

    if "prog" not in _CACHE:
        _CACHE["prog"] = _build_program()
    nc = _CACHE["prog"]

    shared, feeds = _prep_host(raw, Wih0, Wih, Whh, bih, bhh, W1, b1, W2, b2)
    in_maps = [dict(shared, **feeds[c]) for c in range(NCORES)]
    res = run_bass_kernel_spmd(nc, in_maps, list(range(NCORES)),
                               **_CACHE.get("run_kwargs", {}))
    _CACHE["last_results"] = res
    outs = [np.asarray(res.results[c]["out"], np.float32) for c in range(NCORES)]
    full = np.concatenate(outs, axis=1)        # (8, 400)
    return np.ascontiguousarray(full.T).reshape(B, KSEQ, OUT).astype(np.float32)
